# revision 17
# baseline (speedup 1.0000x reference)
"""Distributed Trainium2 Bass kernel for a single attention head.

Reference computation (fp32 jax):
    q = queries @ Wq.T + bq        # [B,S,Df]
    k = keys    @ Wk.T + bk
    v = values  @ Wv.T + bv
    attn = softmax((q @ k.T) / sqrt(Df), axis=-1)
    out  = attn @ v                # [B,S,Df]

with B=4, S=4096, D_MODEL=1024, D_FEATURE=64.

Sharding: 8 cores = (batch b in 0..3) x (query-half h in 0..1).
Core c handles batch b=c//2, q rows [h*2048, (h+1)*2048). Each core gets
its q-half plus the FULL keys/values of its batch (no collectives), all
pre-transposed on the host to m-contraction-major layout and converted
to bf16 so matmuls run at full PE rate and DMA bytes are halved.

Kernel structure (per core):
  - inputs arrive i-block-major: [128, nblk * (8 m-chunks * 512 cols)]
    so each 512-column projection block is one contiguous 1MB DMA and
    projection of block i can start as soon as its DMA lands.
  - projections: psum[64, 512] accumulated over 8 m-chunks,
    lhsT = wT chunk [128, 64], rhs = xT chunk [128, 512]; DVE evicts
    psum -> bf16 SBUF with the per-feature bias added.
  - scores are computed TRANSPOSED, flash-style: ST[j, i]
    (lhsT = kT[64, jc*128:...], rhs = qT[64, i-chunk]) so softmax-exp
    input and the attn@v moving operand are both natural layout.
  - attention runs in TWO i-passes of 1024 q-rows each so that the
    scores psum can double-buffer (2x[128,1024] = 4 banks) next to the
    out.T accumulator ([65,1024] = 2 banks) within the 8 PSUM banks.
    Pass A is interleaved with the k/v projections at k-block
    granularity so the ACT engine starts exp-ing ~8us into the kernel.
  - exp is fused with the 1/8 score scale on ACT; output PT is bf16.
  - attn@v accumulates out.T[f, i] with lhsT = v_aug[j, 65] (v in
    natural [j, f] layout + ones column -> row 64 of out.T is the
    softmax denominator for free).
  - finals per pass: evict out.T, PE-transpose 128-row chunks,
    reciprocal of the denominator column, scale, DMA out fp32 rows.
"""

import numpy as np
import ml_dtypes

import concourse.bass as bass
import concourse.mybir as mybir
import concourse.tile as tile
from concourse import bacc
from concourse.bass_utils import run_bass_kernel_spmd
from concourse.masks import make_identity

B = 4
S = 4096
DM = 1024
DF = 64
NCORES = 8
SQ = S // 2          # local q rows per core
MC = DM // 128       # 8 contraction chunks
NI = 512             # moving-operand tile (one PSUM bank of fp32)
JC = S // 128        # 32 key chunks
NBQ = SQ // NI       # 4 q column blocks
NBK = S // NI        # 8 k/v column blocks
IP = SQ // 2         # 1024: i-rows per attention pass
BF16 = mybir.dt.bfloat16
F32 = mybir.dt.float32
NP_BF16 = ml_dtypes.bfloat16
EXP = mybir.ActivationFunctionType.Exp


def build_kernel(tc):
    nc = tc.nc
    xq = nc.dram_tensor("xq", [128, NBQ * MC * NI], BF16, kind="ExternalInput")
    xk = nc.dram_tensor("xk", [128, NBK * MC * NI], BF16, kind="ExternalInput")
    xv = nc.dram_tensor("xv", [128, NBK * MC * NI], BF16, kind="ExternalInput")
    wT = nc.dram_tensor("wT", [128, MC * 3 * DF], BF16, kind="ExternalInput")
    bias = nc.dram_tensor("bias", [DF, 3], F32, kind="ExternalInput")
    out = nc.dram_tensor("out", [SQ, DF], F32, kind="ExternalOutput")

    from contextlib import ExitStack

    with ExitStack() as ctx:
        const_pool = ctx.enter_context(tc.tile_pool(name="const", bufs=1))
        xin_pool = ctx.enter_context(tc.tile_pool(name="xin", bufs=10))
        act_pool = ctx.enter_context(tc.tile_pool(name="act", bufs=1))
        pt_pool = ctx.enter_context(tc.tile_pool(name="pt", bufs=3))
        outT_pool = ctx.enter_context(tc.tile_pool(name="outT", bufs=1))
        fin_pool = ctx.enter_context(tc.tile_pool(name="fin", bufs=2))
        # PSUM budget (8 banks): ppsum 2x[64,512] = 2 banks (proj psum,
        # also vtrans/finals scratch), spsum 2x[128,1024] = 4 banks
        # (scores double-buffer), opsum 2 banks (warmup scratch, then the
        # per-pass [65,1024] out.T accumulator).
        ppsum = ctx.enter_context(tc.tile_pool(name="ppsum", bufs=2, space="PSUM"))
        spsum = ctx.enter_context(tc.tile_pool(name="spsum", bufs=2, space="PSUM"))
        opsum = ctx.enter_context(tc.tile_pool(name="opsum", bufs=1, space="PSUM"))

        # ---- constants (DMA'd first) ----
        wT_sb = const_pool.tile([128, MC * 3 * DF], BF16, tag="wt")
        nc.sync.dma_start(wT_sb[:], wT[:])
        bias_sb = const_pool.tile([DF, 3], F32, tag="bias")
        nc.sync.dma_start(bias_sb[:], bias[:])
        # preload the ACT exp table while DMAs stream
        scratch = const_pool.tile([DF, 1], F32, tag="scratch")
        nc.scalar.activation(scratch[:], bias_sb[:, 0:1], EXP)
        ident = const_pool.tile([128, 128], BF16, tag="ident")
        make_identity(nc, ident[:])
        identf = const_pool.tile([128, 128], F32, tag="identf")
        make_identity(nc, identf[:])

        # ---- PE warm-up: ~7us of dummy matmuls so the HAM clock gate
        # opens (1.2 -> 2.4 GHz) and stays open until the first real
        # matmul's input DMA lands ----
        warm = opsum.tile([DF, 128], F32, tag="po")
        for _ in range(96):
            nc.tensor.matmul(warm[:], ident[:, 0:DF], ident[:], start=True, stop=True)

        # ---- input DMAs, interleaved q first then k/v alternating ----
        def load_block(x_dram, i):
            t = xin_pool.tile([128, MC * NI], BF16, tag="xin")
            nc.sync.dma_start(t[:], x_dram[:, i * MC * NI:(i + 1) * MC * NI])
            return t

        q_tiles = [load_block(xq, i) for i in range(NBQ)]
        kv_tiles = {}
        for i in range(NBK):
            kv_tiles[("k", i)] = load_block(xk, i)
            kv_tiles[("v", i)] = load_block(xv, i)

        # ---- persistent activations ----
        qT_sb = act_pool.tile([DF, SQ], BF16, tag="qT")            # [64, 2048]
        kT_sb = act_pool.tile([DF, S], BF16, tag="kT")             # [64, 4096]
        vT_sb = act_pool.tile([DF, S], BF16, tag="vT")
        v_sb = act_pool.tile([128, JC * (DF + 1)], BF16, tag="v")  # [128, 32*65]
        nc.gpsimd.memset(v_sb[:], 1.0)  # col DF of every block stays 1.0

        def w_slice(mc_i, which):
            o = mc_i * 3 * DF + which * DF
            return wT_sb[:, o:o + DF]

        def project_block(x_tile, i, which, dest_sb, bias_col):
            """One 512-column projection block accumulated over 8 m-chunks."""
            ps = ppsum.tile([DF, NI], F32, tag="ps")
            for mc_i in range(MC):
                nc.tensor.matmul(
                    ps[:], w_slice(mc_i, which), x_tile[:, mc_i * NI:(mc_i + 1) * NI],
                    start=(mc_i == 0), stop=(mc_i == MC - 1),
                )
            nc.vector.tensor_scalar_add(
                dest_sb[:, i * NI:(i + 1) * NI], ps[:], bias_sb[:, bias_col:bias_col + 1])

        # ---- q projection up front ----
        for i in range(NBQ):
            project_block(q_tiles[i], i, 0, qT_sb, 0)

        # pass-B exp results are computed during pass A and parked in SBUF
        ptb_sb = act_pool.tile([128, JC * IP], BF16, tag="ptb")  # 8 MB

        def attn_chunk(jc, poA):
            """Scores + exp for BOTH i-halves of one key chunk; attn@v for
            half A immediately (half B's exp output parks in ptb_sb)."""
            for ipass in range(2):
                io = ipass * IP
                ss = spsum.tile([128, IP], F32, tag="ss")
                for ii in range(IP // NI):
                    nc.tensor.matmul(
                        ss[:, ii * NI:(ii + 1) * NI],
                        kT_sb[:, jc * 128:(jc + 1) * 128],
                        qT_sb[:, io + ii * NI:io + (ii + 1) * NI],
                        start=True, stop=True,
                    )
                if ipass == 0:
                    pts = pt_pool.tile([128, IP], BF16, tag="pt")
                else:
                    pts = ptb_sb[:, jc * IP:(jc + 1) * IP]
                nc.scalar.activation(pts[:], ss[:], EXP, scale=0.125)
                if ipass == 0:
                    for ii in range(IP // NI):
                        nc.tensor.matmul(
                            poA[:, ii * NI:(ii + 1) * NI],
                            v_sb[:, jc * (DF + 1):(jc + 1) * (DF + 1)],
                            pts[:, ii * NI:(ii + 1) * NI],
                            start=(jc == 0), stop=(jc == JC - 1),
                        )

        def finals_chunk(ipass, outT_sb, ob, c):
            pf = ppsum.tile([128, DF + 1], F32, tag="ps")
            nc.tensor.transpose(
                pf[:], outT_sb[:, c * 128:(c + 1) * 128],
                identf[0:DF + 1, 0:DF + 1])
            rcp = fin_pool.tile([128, 1], F32, tag="rcp")
            nc.vector.reciprocal(rcp[:], pf[:, DF:DF + 1])
            nc.vector.tensor_scalar_mul(ob[:, c, :], pf[:, 0:DF], rcp[:])

        def finals_store(ipass, ob):
            # one strided DMA for all 1024 rows of this i-half
            nc.sync.dma_start(
                out[ipass * IP:(ipass + 1) * IP, :].rearrange(
                    "(c p) f -> p c f", p=128),
                ob[:])

        # ---- pass A: k/v projection interleaved with scores/exp for both
        # i-halves + attn@v for i-half A ----
        poA = opsum.tile([DF + 1, IP], F32, tag="po")
        for kb in range(NBK):
            project_block(kv_tiles[("k", kb)], kb, 1, kT_sb, 1)
            project_block(kv_tiles[("v", kb)], kb, 2, vT_sb, 2)
            for jc in range(4 * kb, 4 * kb + 4):
                pv = ppsum.tile([128, DF], BF16, tag="ps")
                nc.tensor.transpose(
                    pv[:], vT_sb[:, jc * 128:(jc + 1) * 128], ident[0:DF, 0:DF])
                nc.vector.tensor_copy(
                    v_sb[:, jc * (DF + 1):jc * (DF + 1) + DF], pv[:])
            for jc in range(4 * kb, 4 * kb + 4):
                attn_chunk(jc, poA)

        # ---- pass B: attn@v for i-half B from parked exp outputs.
        # poB lives in a freed scores-pool slot so it can start while
        # pass A's accumulator is still being drained; pass A finals are
        # interleaved to fill PE gaps. ----
        poB = spsum.tile([128, IP], F32, tag="ss")
        outT_A = outT_pool.tile([DF + 1, IP], F32, tag="outT")
        obA = fin_pool.tile([128, IP // 128, DF], F32, tag="ob")
        copied_A = False
        for jc in range(JC):
            for ii in range(IP // NI):
                nc.tensor.matmul(
                    poB[0:DF + 1, ii * NI:(ii + 1) * NI],
                    v_sb[:, jc * (DF + 1):(jc + 1) * (DF + 1)],
                    ptb_sb[:, jc * IP + ii * NI:jc * IP + (ii + 1) * NI],
                    start=(jc == 0), stop=(jc == JC - 1),
                )
            if jc == 0:
                nc.vector.tensor_copy(outT_A[:], poA[:])
                copied_A = True
            elif copied_A and jc % 4 == 3:
                c = jc // 4
                finals_chunk(0, outT_A, obA, c)
        finals_store(0, obA)

        outT_B = outT_pool.tile([DF + 1, IP], F32, tag="outT")
        obB = fin_pool.tile([128, IP // 128, DF], F32, tag="ob")
        nc.vector.tensor_copy(outT_B[:], poB[0:DF + 1, :])
        for c in range(IP // 128):
            finals_chunk(1, outT_B, obB, c)
        finals_store(1, obB)


_COMPILED = None


def get_compiled():
    global _COMPILED
    if _COMPILED is None:
        nc = bacc.Bacc("TRN2", target_bir_lowering=False, debug=False,
                       enable_asserts=False, num_devices=NCORES)
        with tile.TileContext(nc) as tc:
            build_kernel(tc)
        nc.compile()
        _COMPILED = nc
    return _COMPILED


def _to_block_major(xT):
    """[DM, s_len] -> [128, nblk*MC*NI]: 512-col blocks, m-chunk-major inside."""
    s_len = xT.shape[1]
    nblk = s_len // NI
    # (mc, p, blk, s) -> (p, blk, mc, s)
    return np.ascontiguousarray(
        xT.reshape(MC, 128, nblk, NI).transpose(1, 2, 0, 3).reshape(128, nblk * MC * NI))


def make_in_maps(queries, keys, values, Wq, bq, Wk, bk, Wv, bv):
    queries = np.asarray(queries, dtype=np.float32)
    keys = np.asarray(keys, dtype=np.float32)
    values = np.asarray(values, dtype=np.float32)
    wT_full = np.concatenate(
        [np.asarray(Wq).T, np.asarray(Wk).T, np.asarray(Wv).T], axis=1)  # [DM, 192]
    wT_host = np.ascontiguousarray(
        wT_full.reshape(MC, 128, 3 * DF).transpose(1, 0, 2).reshape(128, MC * 3 * DF)
    ).astype(NP_BF16)
    bias_host = np.stack(
        [np.asarray(bq), np.asarray(bk), np.asarray(bv)], axis=1
    ).astype(np.float32)

    in_maps = []
    for c in range(NCORES):
        b, h = c // 2, c % 2
        in_maps.append({
            "xq": _to_block_major(queries[b, h * SQ:(h + 1) * SQ, :].T).astype(NP_BF16),
            "xk": _to_block_major(keys[b].T).astype(NP_BF16),
            "xv": _to_block_major(values[b].T).astype(NP_BF16),
            "wT": wT_host, "bias": bias_host,
        })
    return in_maps


def assemble(results):
    out = np.zeros((B, S, DF), dtype=np.float32)
    for c in range(NCORES):
        b, h = c // 2, c % 2
        out[b, h * SQ:(h + 1) * SQ, :] = results[c]["out"]
    return out


def kernel(**inputs):
    nc = get_compiled()
    in_maps = make_in_maps(**inputs)
    res = run_bass_kernel_spmd(nc, in_maps, core_ids=list(range(NCORES)))
    return assemble(res.results)


# revision 18
# speedup vs baseline: 1.1733x; 1.1733x over previous
"""Distributed Trainium2 Bass kernel for a single attention head.

Reference computation (fp32 jax):
    q = queries @ Wq.T + bq        # [B,S,Df]
    k = keys    @ Wk.T + bk
    v = values  @ Wv.T + bv
    attn = softmax((q @ k.T) / sqrt(Df), axis=-1)
    out  = attn @ v                # [B,S,Df]

with B=4, S=4096, D_MODEL=1024, D_FEATURE=64.

Sharding: 8 cores = (batch b in 0..3) x (query-half h in 0..1).
Core c handles batch b=c//2, q rows [h*2048, (h+1)*2048). Each core gets
its q-half plus the FULL keys/values of its batch (no collectives), all
pre-transposed on the host to m-contraction-major layout and converted
to bf16 so matmuls run at full PE rate and DMA bytes are halved.

Kernel structure (per core):
  - inputs arrive i-block-major: [128, nblk * (8 m-chunks * 512 cols)]
    so each 512-column projection block is one contiguous 1MB DMA and
    projection of block i can start as soon as its DMA lands.
  - projections: psum[64, 512] accumulated over 8 m-chunks,
    lhsT = wT chunk [128, 64], rhs = xT chunk [128, 512]; DVE evicts
    psum -> bf16 SBUF with the per-feature bias added.
  - scores are computed TRANSPOSED, flash-style: ST[j, i]
    (lhsT = kT[64, jc*128:...], rhs = qT[64, i-chunk]) so softmax-exp
    input and the attn@v moving operand are both natural layout.
  - attention runs in TWO i-passes of 1024 q-rows each so that the
    scores psum can double-buffer (2x[128,1024] = 4 banks) next to the
    out.T accumulator ([65,1024] = 2 banks) within the 8 PSUM banks.
    Pass A is interleaved with the k/v projections at k-block
    granularity so the ACT engine starts exp-ing ~8us into the kernel.
  - exp is fused with the 1/8 score scale on ACT; output PT is bf16.
  - attn@v accumulates out.T[f, i] with lhsT = v_aug[j, 65] (v in
    natural [j, f] layout + ones column -> row 64 of out.T is the
    softmax denominator for free).
  - finals per pass: evict out.T, PE-transpose 128-row chunks,
    reciprocal of the denominator column, scale, DMA out fp32 rows.
"""

import numpy as np
import ml_dtypes

import concourse.bass as bass
import concourse.mybir as mybir
import concourse.tile as tile
from concourse import bacc
from concourse.bass_utils import run_bass_kernel_spmd
from concourse.masks import make_identity

B = 4
S = 4096
DM = 1024
DF = 64
NCORES = 8
SQ = S // 2          # local q rows per core
MC = DM // 128       # 8 contraction chunks
NI = 512             # moving-operand tile (one PSUM bank of fp32)
JC = S // 128        # 32 key chunks
NBQ = SQ // NI       # 4 q column blocks
NBK = S // NI        # 8 k/v column blocks
IP = SQ // 2         # 1024: i-rows per attention pass
BF16 = mybir.dt.bfloat16
F32 = mybir.dt.float32
NP_BF16 = ml_dtypes.bfloat16
EXP = mybir.ActivationFunctionType.Exp


def build_kernel(tc):
    nc = tc.nc
    xq = nc.dram_tensor("xq", [128, NBQ * MC * NI], BF16, kind="ExternalInput")
    xk = nc.dram_tensor("xk", [128, NBK * MC * NI], BF16, kind="ExternalInput")
    xv = nc.dram_tensor("xv", [128, NBK * MC * NI], BF16, kind="ExternalInput")
    wT = nc.dram_tensor("wT", [128, MC * 3 * DF], BF16, kind="ExternalInput")
    bias = nc.dram_tensor("bias", [DF, 3], F32, kind="ExternalInput")
    out = nc.dram_tensor("out", [SQ, DF], F32, kind="ExternalOutput")

    from contextlib import ExitStack

    with ExitStack() as ctx:
        const_pool = ctx.enter_context(tc.tile_pool(name="const", bufs=1))
        xin_pool = ctx.enter_context(tc.tile_pool(name="xin", bufs=10))
        act_pool = ctx.enter_context(tc.tile_pool(name="act", bufs=1))
        pt_pool = ctx.enter_context(tc.tile_pool(name="pt", bufs=3))
        outT_pool = ctx.enter_context(tc.tile_pool(name="outT", bufs=1))
        fin_pool = ctx.enter_context(tc.tile_pool(name="fin", bufs=2))
        # PSUM budget (8 banks): ppsum 2x[64,512] = 2 banks (proj psum,
        # also vtrans/finals scratch), spsum 2x[128,1024] = 4 banks
        # (scores double-buffer), opsum 2 banks (warmup scratch, then the
        # per-pass [65,1024] out.T accumulator).
        ppsum = ctx.enter_context(tc.tile_pool(name="ppsum", bufs=2, space="PSUM"))
        spsum = ctx.enter_context(tc.tile_pool(name="spsum", bufs=2, space="PSUM"))
        opsum = ctx.enter_context(tc.tile_pool(name="opsum", bufs=1, space="PSUM"))

        # ---- constants (DMA'd first) ----
        wT_sb = const_pool.tile([128, MC * 3 * DF], BF16, tag="wt")
        nc.sync.dma_start(wT_sb[:], wT[:])
        bias_sb = const_pool.tile([DF, 3], F32, tag="bias")
        nc.sync.dma_start(bias_sb[:], bias[:])
        # preload the ACT exp table while DMAs stream
        scratch = const_pool.tile([DF, 1], F32, tag="scratch")
        nc.scalar.activation(scratch[:], bias_sb[:, 0:1], EXP)
        ident = const_pool.tile([128, 128], BF16, tag="ident")
        make_identity(nc, ident[:])
        identf = const_pool.tile([128, 128], F32, tag="identf")
        make_identity(nc, identf[:])

        # ---- PE warm-up: ~7us of dummy matmuls so the HAM clock gate
        # opens (1.2 -> 2.4 GHz) and stays open until the first real
        # matmul's input DMA lands ----
        warm = opsum.tile([DF, 128], F32, tag="po")
        for _ in range(96):
            nc.tensor.matmul(warm[:], ident[:, 0:DF], ident[:], start=True, stop=True)

        # ---- input DMAs, interleaved q first then k/v alternating ----
        def load_block(x_dram, i):
            t = xin_pool.tile([128, MC * NI], BF16, tag="xin")
            nc.sync.dma_start(t[:], x_dram[:, i * MC * NI:(i + 1) * MC * NI])
            return t

        q_tiles = [load_block(xq, i) for i in range(NBQ)]
        kv_tiles = {}
        for i in range(NBK):
            kv_tiles[("k", i)] = load_block(xk, i)
            kv_tiles[("v", i)] = load_block(xv, i)

        # ---- persistent activations ----
        qT_sb = act_pool.tile([DF, SQ], BF16, tag="qT")            # [64, 2048]
        kT_sb = act_pool.tile([DF, S], BF16, tag="kT")             # [64, 4096]
        vT_sb = act_pool.tile([DF, S], BF16, tag="vT")
        v_sb = act_pool.tile([128, JC * (DF + 1)], BF16, tag="v")  # [128, 32*65]
        nc.gpsimd.memset(v_sb[:], 1.0)  # col DF of every block stays 1.0

        def w_slice(mc_i, which):
            o = mc_i * 3 * DF + which * DF
            return wT_sb[:, o:o + DF]

        def project_block(x_tile, i, which, dest_sb, bias_col):
            """One 512-column projection block accumulated over 8 m-chunks."""
            ps = ppsum.tile([DF, NI], F32, tag="ps")
            for mc_i in range(MC):
                nc.tensor.matmul(
                    ps[:], w_slice(mc_i, which), x_tile[:, mc_i * NI:(mc_i + 1) * NI],
                    start=(mc_i == 0), stop=(mc_i == MC - 1),
                )
            nc.vector.tensor_scalar_add(
                dest_sb[:, i * NI:(i + 1) * NI], ps[:], bias_sb[:, bias_col:bias_col + 1])

        # ---- q projection up front ----
        for i in range(NBQ):
            project_block(q_tiles[i], i, 0, qT_sb, 0)

        # pass-B exp results are computed during pass A and parked in SBUF
        ptb_sb = act_pool.tile([128, JC * IP], BF16, tag="ptb")  # 8 MB

        def attn_chunk(jc, poA):
            """Scores + exp for BOTH i-halves of one key chunk; attn@v for
            half A immediately (half B's exp output parks in ptb_sb)."""
            for ipass in range(2):
                io = ipass * IP
                ss = spsum.tile([128, IP], F32, tag="ss")
                for ii in range(IP // NI):
                    nc.tensor.matmul(
                        ss[:, ii * NI:(ii + 1) * NI],
                        kT_sb[:, jc * 128:(jc + 1) * 128],
                        qT_sb[:, io + ii * NI:io + (ii + 1) * NI],
                        start=True, stop=True,
                    )
                if ipass == 0:
                    pts = pt_pool.tile([128, IP], BF16, tag="pt")
                else:
                    pts = ptb_sb[:, jc * IP:(jc + 1) * IP]
                nc.scalar.activation(pts[:], ss[:], EXP, scale=0.125)
                if ipass == 0:
                    for ii in range(IP // NI):
                        nc.tensor.matmul(
                            poA[:, ii * NI:(ii + 1) * NI],
                            v_sb[:, jc * (DF + 1):(jc + 1) * (DF + 1)],
                            pts[:, ii * NI:(ii + 1) * NI],
                            start=(jc == 0), stop=(jc == JC - 1),
                        )

        def finals_chunk(ipass, outT_sb, ob, c):
            pf = ppsum.tile([128, DF + 1], F32, tag="ps")
            nc.tensor.transpose(
                pf[:], outT_sb[:, c * 128:(c + 1) * 128],
                identf[0:DF + 1, 0:DF + 1])
            rcp = fin_pool.tile([128, 1], F32, tag="rcp")
            nc.vector.reciprocal(rcp[:], pf[:, DF:DF + 1])
            nc.vector.tensor_scalar_mul(ob[:, c, :], pf[:, 0:DF], rcp[:])

        def finals_store(ipass, ob):
            # one strided DMA for all 1024 rows of this i-half
            nc.sync.dma_start(
                out[ipass * IP:(ipass + 1) * IP, :].rearrange(
                    "(c p) f -> p c f", p=128),
                ob[:])

        # ---- pass A: k/v projection interleaved with scores/exp for both
        # i-halves + attn@v for i-half A ----
        poA = opsum.tile([DF + 1, IP], F32, tag="po")
        for kb in range(NBK):
            project_block(kv_tiles[("k", kb)], kb, 1, kT_sb, 1)
            project_block(kv_tiles[("v", kb)], kb, 2, vT_sb, 2)
            for jc in range(4 * kb, 4 * kb + 4):
                pv = ppsum.tile([128, DF], BF16, tag="ps")
                nc.tensor.transpose(
                    pv[:], vT_sb[:, jc * 128:(jc + 1) * 128], ident[0:DF, 0:DF])
                nc.vector.tensor_copy(
                    v_sb[:, jc * (DF + 1):jc * (DF + 1) + DF], pv[:])
            for jc in range(4 * kb, 4 * kb + 4):
                attn_chunk(jc, poA)

        # ---- pass B: attn@v for i-half B from parked exp outputs; pass A
        # finals are interleaved to fill PE gaps. ----
        outT_A = outT_pool.tile([DF + 1, IP], F32, tag="outT")
        obA = fin_pool.tile([128, IP // 128, DF], F32, tag="ob")
        nc.vector.tensor_copy(outT_A[:], poA[:])
        poB = opsum.tile([DF + 1, IP], F32, tag="po")
        for jc in range(JC):
            for ii in range(IP // NI):
                nc.tensor.matmul(
                    poB[:, ii * NI:(ii + 1) * NI],
                    v_sb[:, jc * (DF + 1):(jc + 1) * (DF + 1)],
                    ptb_sb[:, jc * IP + ii * NI:jc * IP + (ii + 1) * NI],
                    start=(jc == 0), stop=(jc == JC - 1),
                )
            if jc % 4 == 3:
                finals_chunk(0, outT_A, obA, jc // 4)
        finals_store(0, obA)

        outT_B = outT_pool.tile([DF + 1, IP], F32, tag="outT")
        obB = fin_pool.tile([128, IP // 128, DF], F32, tag="ob")
        nc.vector.tensor_copy(outT_B[:], poB[:])
        for c in range(IP // 128):
            finals_chunk(1, outT_B, obB, c)
        finals_store(1, obB)


_COMPILED = None


def get_compiled():
    global _COMPILED
    if _COMPILED is None:
        nc = bacc.Bacc("TRN2", target_bir_lowering=False, debug=False,
                       enable_asserts=False, num_devices=NCORES)
        with tile.TileContext(nc) as tc:
            build_kernel(tc)
        nc.compile()
        _COMPILED = nc
    return _COMPILED


def _to_block_major(xT):
    """[DM, s_len] -> [128, nblk*MC*NI]: 512-col blocks, m-chunk-major inside."""
    s_len = xT.shape[1]
    nblk = s_len // NI
    # (mc, p, blk, s) -> (p, blk, mc, s)
    return np.ascontiguousarray(
        xT.reshape(MC, 128, nblk, NI).transpose(1, 2, 0, 3).reshape(128, nblk * MC * NI))


def make_in_maps(queries, keys, values, Wq, bq, Wk, bk, Wv, bv):
    queries = np.asarray(queries, dtype=np.float32)
    keys = np.asarray(keys, dtype=np.float32)
    values = np.asarray(values, dtype=np.float32)
    wT_full = np.concatenate(
        [np.asarray(Wq).T, np.asarray(Wk).T, np.asarray(Wv).T], axis=1)  # [DM, 192]
    wT_host = np.ascontiguousarray(
        wT_full.reshape(MC, 128, 3 * DF).transpose(1, 0, 2).reshape(128, MC * 3 * DF)
    ).astype(NP_BF16)
    bias_host = np.stack(
        [np.asarray(bq), np.asarray(bk), np.asarray(bv)], axis=1
    ).astype(np.float32)

    in_maps = []
    for c in range(NCORES):
        b, h = c // 2, c % 2
        in_maps.append({
            "xq": _to_block_major(queries[b, h * SQ:(h + 1) * SQ, :].T).astype(NP_BF16),
            "xk": _to_block_major(keys[b].T).astype(NP_BF16),
            "xv": _to_block_major(values[b].T).astype(NP_BF16),
            "wT": wT_host, "bias": bias_host,
        })
    return in_maps


def assemble(results):
    out = np.zeros((B, S, DF), dtype=np.float32)
    for c in range(NCORES):
        b, h = c // 2, c % 2
        out[b, h * SQ:(h + 1) * SQ, :] = results[c]["out"]
    return out


def kernel(**inputs):
    nc = get_compiled()
    in_maps = make_in_maps(**inputs)
    res = run_bass_kernel_spmd(nc, in_maps, core_ids=list(range(NCORES)))
    return assemble(res.results)


# revision 20
# speedup vs baseline: 1.2092x; 1.0305x over previous
"""Distributed Trainium2 Bass kernel for a single attention head.

Reference computation (fp32 jax):
    q = queries @ Wq.T + bq        # [B,S,Df]
    k = keys    @ Wk.T + bk
    v = values  @ Wv.T + bv
    attn = softmax((q @ k.T) / sqrt(Df), axis=-1)
    out  = attn @ v                # [B,S,Df]

with B=4, S=4096, D_MODEL=1024, D_FEATURE=64.

Sharding: 8 cores = (batch b in 0..3) x (query-half h in 0..1).
Core c handles batch b=c//2, q rows [h*2048, (h+1)*2048). Each core gets
its q-half plus the FULL keys/values of its batch (no collectives), all
pre-transposed on the host to m-contraction-major layout and converted
to bf16 so matmuls run at full PE rate and DMA bytes are halved.

Kernel structure (per core):
  - inputs arrive i-block-major: [128, nblk * (8 m-chunks * 512 cols)]
    so each 512-column projection block is one contiguous 1MB DMA and
    projection of block i can start as soon as its DMA lands.
  - projections: psum[64, 512] accumulated over 8 m-chunks,
    lhsT = wT chunk [128, 64], rhs = xT chunk [128, 512]; DVE evicts
    psum -> bf16 SBUF with the per-feature bias added.
  - scores are computed TRANSPOSED, flash-style: ST[j, i]
    (lhsT = kT[64, jc*128:...], rhs = qT[64, i-chunk]) so softmax-exp
    input and the attn@v moving operand are both natural layout.
  - attention runs in TWO i-passes of 1024 q-rows each so that the
    scores psum can double-buffer (2x[128,1024] = 4 banks) next to the
    out.T accumulator ([65,1024] = 2 banks) within the 8 PSUM banks.
    Pass A is interleaved with the k/v projections at k-block
    granularity so the ACT engine starts exp-ing ~8us into the kernel.
  - exp is fused with the 1/8 score scale on ACT; output PT is bf16.
  - attn@v accumulates out.T[f, i] with lhsT = v_aug[j, 65] (v in
    natural [j, f] layout + ones column -> row 64 of out.T is the
    softmax denominator for free).
  - finals per pass: evict out.T, PE-transpose 128-row chunks,
    reciprocal of the denominator column, scale, DMA out fp32 rows.
"""

import numpy as np
import ml_dtypes

import concourse.bass as bass
import concourse.mybir as mybir
import concourse.tile as tile
from concourse import bacc
from concourse.bass_utils import run_bass_kernel_spmd
from concourse.masks import make_identity

B = 4
S = 4096
DM = 1024
DF = 64
NCORES = 8
SQ = S // 2          # local q rows per core
MC = DM // 128       # 8 contraction chunks
NI = 512             # moving-operand tile (one PSUM bank of fp32)
JC = S // 128        # 32 key chunks
NBQ = SQ // NI       # 4 q column blocks
NBK = S // NI        # 8 k/v column blocks
IP = SQ // 2         # 1024: i-rows per attention pass
WB = 5 * DF          # per-m-chunk weight columns: [wq|wq|wk|wk|wv]
BF16 = mybir.dt.bfloat16
F32 = mybir.dt.float32
NP_BF16 = ml_dtypes.bfloat16
EXP = mybir.ActivationFunctionType.Exp


def build_kernel(tc):
    nc = tc.nc
    xq = nc.dram_tensor("xq", [128, NBQ * MC * NI], BF16, kind="ExternalInput")
    xk = nc.dram_tensor("xk", [128, NBK * MC * NI], BF16, kind="ExternalInput")
    xv = nc.dram_tensor("xv", [128, NBK * MC * NI], BF16, kind="ExternalInput")
    wT = nc.dram_tensor("wT", [128, MC * WB], BF16, kind="ExternalInput")
    bias = nc.dram_tensor("bias", [128, 3], F32, kind="ExternalInput")
    out = nc.dram_tensor("out", [SQ, DF], F32, kind="ExternalOutput")

    from contextlib import ExitStack

    with ExitStack() as ctx:
        const_pool = ctx.enter_context(tc.tile_pool(name="const", bufs=1))
        xin_pool = ctx.enter_context(tc.tile_pool(name="xin", bufs=10))
        act_pool = ctx.enter_context(tc.tile_pool(name="act", bufs=1))
        pt_pool = ctx.enter_context(tc.tile_pool(name="pt", bufs=3))
        outT_pool = ctx.enter_context(tc.tile_pool(name="outT", bufs=1))
        fin_pool = ctx.enter_context(tc.tile_pool(name="fin", bufs=2))
        # PSUM budget (8 banks): ppsum 2x[64,512] = 2 banks (proj psum,
        # also vtrans/finals scratch), spsum 2x[128,1024] = 4 banks
        # (scores double-buffer), opsum 2 banks (warmup scratch, then the
        # per-pass [65,1024] out.T accumulator).
        ppsum = ctx.enter_context(tc.tile_pool(name="ppsum", bufs=2, space="PSUM"))
        spsum = ctx.enter_context(tc.tile_pool(name="spsum", bufs=2, space="PSUM"))
        opsum = ctx.enter_context(tc.tile_pool(name="opsum", bufs=1, space="PSUM"))

        # ---- constants (DMA'd first) ----
        wT_sb = const_pool.tile([128, MC * WB], BF16, tag="wt")
        nc.sync.dma_start(wT_sb[:], wT[:])
        bias_sb = const_pool.tile([128, 3], F32, tag="bias")
        nc.sync.dma_start(bias_sb[:], bias[:])
        # preload the ACT exp table while DMAs stream
        scratch = const_pool.tile([DF, 1], F32, tag="scratch")
        nc.scalar.activation(scratch[:], bias_sb[0:DF, 0:1], EXP)
        ident = const_pool.tile([128, 128], BF16, tag="ident")
        make_identity(nc, ident[:])
        identf = const_pool.tile([128, 128], F32, tag="identf")
        make_identity(nc, identf[:])

        # ---- PE warm-up: ~7us of dummy matmuls so the HAM clock gate
        # opens (1.2 -> 2.4 GHz) and stays open until the first real
        # matmul's input DMA lands ----
        warm = opsum.tile([DF, 128], F32, tag="po")
        for _ in range(96):
            nc.tensor.matmul(warm[:], ident[:, 0:DF], ident[:], start=True, stop=True)

        # ---- input DMAs, interleaved q first then k/v alternating ----
        def load_block(x_dram, i):
            t = xin_pool.tile([128, MC * NI], BF16, tag="xin")
            nc.sync.dma_start(t[:], x_dram[:, i * MC * NI:(i + 1) * MC * NI])
            return t

        q_tiles = [load_block(xq, i) for i in range(NBQ)]
        kv_tiles = {}
        for i in range(NBK):
            kv_tiles[("k", i)] = load_block(xk, i)
            kv_tiles[("v", i)] = load_block(xv, i)

        # ---- persistent activations ----
        # q/k projections land duplicated in both partition halves so the
        # score matmuls can run pair-wise on independent 64-row PE tiles
        qT_sb = act_pool.tile([128, SQ], BF16, tag="qT")
        kT_sb = act_pool.tile([128, S], BF16, tag="kT")
        vT_sb = act_pool.tile([DF, S], BF16, tag="vT")
        v_sb = act_pool.tile([128, JC * (DF + 1)], BF16, tag="v")  # [128, 32*65]
        nc.gpsimd.memset(v_sb[:], 1.0)  # col DF of every block stays 1.0

        def w_slice(mc_i, which):
            # which: 0 = [wq|wq], 1 = [wk|wk] (128-wide dup), 2 = wv (64)
            o = mc_i * WB + which * 2 * DF
            return wT_sb[:, o:o + (2 * DF if which < 2 else DF)]

        def project_block(x_tile, i, which, dest_sb, bias_col):
            """One 512-column projection block accumulated over 8 m-chunks."""
            rows = 2 * DF if which < 2 else DF
            ps = ppsum.tile([rows, NI], F32, tag="ps")
            for mc_i in range(MC):
                nc.tensor.matmul(
                    ps[:], w_slice(mc_i, which), x_tile[:, mc_i * NI:(mc_i + 1) * NI],
                    start=(mc_i == 0), stop=(mc_i == MC - 1),
                )
            nc.vector.tensor_scalar_add(
                dest_sb[:, i * NI:(i + 1) * NI], ps[:],
                bias_sb[0:rows, bias_col:bias_col + 1])

        # ---- q projection up front ----
        for i in range(NBQ):
            project_block(q_tiles[i], i, 0, qT_sb, 0)

        # pass-B exp results are computed during pass A and parked in SBUF
        ptb_sb = act_pool.tile([128, JC * IP], BF16, tag="ptb")  # 8 MB

        def attn_pair(jc0, poA):
            """Scores + exp for BOTH i-halves of TWO key chunks; the two
            chunks' score matmuls run on independent 64-row PE tiles
            (partitions 0-63 / 64-127 of the duplicated qT/kT), so they
            stream concurrently. attn@v for i-half A follows immediately;
            i-half B's exp output parks in ptb_sb."""
            for ipass in range(2):
                io = ipass * IP
                ss0 = spsum.tile([128, IP], F32, tag="ss", name="ss0")
                ss1 = spsum.tile([128, IP], F32, tag="ss", name="ss1")
                sss = [ss0, ss1]
                for ii in range(IP // NI):
                    for t in range(2):
                        jc = jc0 + t
                        p0 = t * DF
                        nc.tensor.matmul(
                            sss[t][:, ii * NI:(ii + 1) * NI],
                            kT_sb[p0:p0 + DF, jc * 128:(jc + 1) * 128],
                            qT_sb[p0:p0 + DF, io + ii * NI:io + (ii + 1) * NI],
                            start=True, stop=True,
                        )
                for t in range(2):
                    jc = jc0 + t
                    if ipass == 0:
                        pts = pt_pool.tile([128, IP], BF16, tag="pt")
                    else:
                        pts = ptb_sb[:, jc * IP:(jc + 1) * IP]
                    nc.scalar.activation(pts[:], sss[t][:], EXP, scale=0.125)
                    if ipass == 0:
                        for ii in range(IP // NI):
                            nc.tensor.matmul(
                                poA[:, ii * NI:(ii + 1) * NI],
                                v_sb[:, jc * (DF + 1):(jc + 1) * (DF + 1)],
                                pts[:, ii * NI:(ii + 1) * NI],
                                start=(jc == 0), stop=(jc == JC - 1),
                            )

        def finals_chunk(ipass, outT_sb, ob, c):
            pf = ppsum.tile([128, DF + 1], F32, tag="ps")
            nc.tensor.transpose(
                pf[:], outT_sb[:, c * 128:(c + 1) * 128],
                identf[0:DF + 1, 0:DF + 1])
            rcp = fin_pool.tile([128, 1], F32, tag="rcp")
            nc.vector.reciprocal(rcp[:], pf[:, DF:DF + 1])
            nc.vector.tensor_scalar_mul(ob[:, c, :], pf[:, 0:DF], rcp[:])

        def finals_store(ipass, ob):
            # one strided DMA for all 1024 rows of this i-half
            nc.sync.dma_start(
                out[ipass * IP:(ipass + 1) * IP, :].rearrange(
                    "(c p) f -> p c f", p=128),
                ob[:])

        # ---- pass A: k/v projection interleaved with scores/exp for both
        # i-halves + attn@v for i-half A ----
        poA = opsum.tile([DF + 1, IP], F32, tag="po")
        for kb in range(NBK):
            project_block(kv_tiles[("k", kb)], kb, 1, kT_sb, 1)
            project_block(kv_tiles[("v", kb)], kb, 2, vT_sb, 2)
            for jc in range(4 * kb, 4 * kb + 4):
                pv = ppsum.tile([128, DF], BF16, tag="ps")
                nc.tensor.transpose(
                    pv[:], vT_sb[:, jc * 128:(jc + 1) * 128], ident[0:DF, 0:DF])
                nc.vector.tensor_copy(
                    v_sb[:, jc * (DF + 1):jc * (DF + 1) + DF], pv[:])
            for jc0 in range(4 * kb, 4 * kb + 4, 2):
                attn_pair(jc0, poA)

        # ---- pass B: attn@v for i-half B from parked exp outputs; pass A
        # finals are interleaved to fill PE gaps. ----
        outT_A = outT_pool.tile([DF + 1, IP], F32, tag="outT")
        obA = fin_pool.tile([128, IP // 128, DF], F32, tag="ob")
        nc.vector.tensor_copy(outT_A[:], poA[:])
        poB = opsum.tile([DF + 1, IP], F32, tag="po")
        for jc in range(JC):
            for ii in range(IP // NI):
                nc.tensor.matmul(
                    poB[:, ii * NI:(ii + 1) * NI],
                    v_sb[:, jc * (DF + 1):(jc + 1) * (DF + 1)],
                    ptb_sb[:, jc * IP + ii * NI:jc * IP + (ii + 1) * NI],
                    start=(jc == 0), stop=(jc == JC - 1),
                )
            if jc % 4 == 3:
                finals_chunk(0, outT_A, obA, jc // 4)
        finals_store(0, obA)

        outT_B = outT_pool.tile([DF + 1, IP], F32, tag="outT")
        obB = fin_pool.tile([128, IP // 128, DF], F32, tag="ob")
        nc.vector.tensor_copy(outT_B[:], poB[:])
        for c in range(IP // 128):
            finals_chunk(1, outT_B, obB, c)
        finals_store(1, obB)


_COMPILED = None


def get_compiled():
    global _COMPILED
    if _COMPILED is None:
        nc = bacc.Bacc("TRN2", target_bir_lowering=False, debug=False,
                       enable_asserts=False, num_devices=NCORES)
        with tile.TileContext(nc) as tc:
            build_kernel(tc)
        nc.compile()
        _COMPILED = nc
    return _COMPILED


def _to_block_major(xT):
    """[DM, s_len] -> [128, nblk*MC*NI]: 512-col blocks, m-chunk-major inside."""
    s_len = xT.shape[1]
    nblk = s_len // NI
    # (mc, p, blk, s) -> (p, blk, mc, s)
    return np.ascontiguousarray(
        xT.reshape(MC, 128, nblk, NI).transpose(1, 2, 0, 3).reshape(128, nblk * MC * NI))


def make_in_maps(queries, keys, values, Wq, bq, Wk, bk, Wv, bv):
    queries = np.asarray(queries, dtype=np.float32)
    keys = np.asarray(keys, dtype=np.float32)
    values = np.asarray(values, dtype=np.float32)
    WqT, WkT, WvT = np.asarray(Wq).T, np.asarray(Wk).T, np.asarray(Wv).T
    wT_full = np.concatenate([WqT, WqT, WkT, WkT, WvT], axis=1)  # [DM, 320]
    wT_host = np.ascontiguousarray(
        wT_full.reshape(MC, 128, WB).transpose(1, 0, 2).reshape(128, MC * WB)
    ).astype(NP_BF16)
    bias64 = np.stack(
        [np.asarray(bq), np.asarray(bk), np.asarray(bv)], axis=1
    ).astype(np.float32)
    bias_host = np.concatenate([bias64, bias64], axis=0)  # [128, 3]

    in_maps = []
    for c in range(NCORES):
        b, h = c // 2, c % 2
        in_maps.append({
            "xq": _to_block_major(queries[b, h * SQ:(h + 1) * SQ, :].T).astype(NP_BF16),
            "xk": _to_block_major(keys[b].T).astype(NP_BF16),
            "xv": _to_block_major(values[b].T).astype(NP_BF16),
            "wT": wT_host, "bias": bias_host,
        })
    return in_maps


def assemble(results):
    out = np.zeros((B, S, DF), dtype=np.float32)
    for c in range(NCORES):
        b, h = c // 2, c % 2
        out[b, h * SQ:(h + 1) * SQ, :] = results[c]["out"]
    return out


def kernel(**inputs):
    nc = get_compiled()
    in_maps = make_in_maps(**inputs)
    res = run_bass_kernel_spmd(nc, in_maps, core_ids=list(range(NCORES)))
    return assemble(res.results)


# revision 21
# speedup vs baseline: 1.2146x; 1.0045x over previous
"""Distributed Trainium2 Bass kernel for a single attention head.

Reference computation (fp32 jax):
    q = queries @ Wq.T + bq        # [B,S,Df]
    k = keys    @ Wk.T + bk
    v = values  @ Wv.T + bv
    attn = softmax((q @ k.T) / sqrt(Df), axis=-1)
    out  = attn @ v                # [B,S,Df]

with B=4, S=4096, D_MODEL=1024, D_FEATURE=64.

Sharding: 8 cores = (batch b in 0..3) x (query-half h in 0..1).
Core c handles batch b=c//2, q rows [h*2048, (h+1)*2048). Each core gets
its q-half plus the FULL keys/values of its batch (no collectives), all
pre-transposed on the host to m-contraction-major layout and converted
to bf16 so matmuls run at full PE rate and DMA bytes are halved.

Kernel structure (per core):
  - inputs arrive i-block-major: [128, nblk * (8 m-chunks * 512 cols)]
    so each 512-column projection block is one contiguous 1MB DMA and
    projection of block i can start as soon as its DMA lands.
  - projections: psum[64, 512] accumulated over 8 m-chunks,
    lhsT = wT chunk [128, 64], rhs = xT chunk [128, 512]; DVE evicts
    psum -> bf16 SBUF with the per-feature bias added.
  - scores are computed TRANSPOSED, flash-style: ST[j, i]
    (lhsT = kT[64, jc*128:...], rhs = qT[64, i-chunk]) so softmax-exp
    input and the attn@v moving operand are both natural layout.
  - attention runs in TWO i-passes of 1024 q-rows each so that the
    scores psum can double-buffer (2x[128,1024] = 4 banks) next to the
    out.T accumulator ([65,1024] = 2 banks) within the 8 PSUM banks.
    Pass A is interleaved with the k/v projections at k-block
    granularity so the ACT engine starts exp-ing ~8us into the kernel.
  - exp is fused with the 1/8 score scale on ACT; output PT is bf16.
  - attn@v accumulates out.T[f, i] with lhsT = v_aug[j, 65] (v in
    natural [j, f] layout + ones column -> row 64 of out.T is the
    softmax denominator for free).
  - finals per pass: evict out.T, PE-transpose 128-row chunks,
    reciprocal of the denominator column, scale, DMA out fp32 rows.
"""

import numpy as np
import ml_dtypes

import concourse.bass as bass
import concourse.mybir as mybir
import concourse.tile as tile
from concourse import bacc
from concourse.bass_utils import run_bass_kernel_spmd
from concourse.masks import make_identity

B = 4
S = 4096
DM = 1024
DF = 64
NCORES = 8
SQ = S // 2          # local q rows per core
MC = DM // 128       # 8 contraction chunks
NI = 512             # moving-operand tile (one PSUM bank of fp32)
JC = S // 128        # 32 key chunks
NBQ = SQ // NI       # 4 q column blocks
NBK = S // NI        # 8 k/v column blocks
IP = SQ // 2         # 1024: i-rows per attention pass
WB = 5 * DF          # per-m-chunk weight columns: [wq|wq|wk|wk|wv]
BF16 = mybir.dt.bfloat16
F32 = mybir.dt.float32
NP_BF16 = ml_dtypes.bfloat16
EXP = mybir.ActivationFunctionType.Exp


def build_kernel(tc):
    nc = tc.nc
    xq = nc.dram_tensor("xq", [128, NBQ * MC * NI], BF16, kind="ExternalInput")
    xk = nc.dram_tensor("xk", [128, NBK * MC * NI], BF16, kind="ExternalInput")
    xv = nc.dram_tensor("xv", [128, NBK * MC * NI], BF16, kind="ExternalInput")
    wT = nc.dram_tensor("wT", [128, MC * WB], BF16, kind="ExternalInput")
    bias = nc.dram_tensor("bias", [128, 3], F32, kind="ExternalInput")
    out = nc.dram_tensor("out", [SQ, DF], F32, kind="ExternalOutput")

    from contextlib import ExitStack

    with ExitStack() as ctx:
        const_pool = ctx.enter_context(tc.tile_pool(name="const", bufs=1))
        xin_pool = ctx.enter_context(tc.tile_pool(name="xin", bufs=9))
        act_pool = ctx.enter_context(tc.tile_pool(name="act", bufs=1))
        pt_pool = ctx.enter_context(tc.tile_pool(name="pt", bufs=4))
        outT_pool = ctx.enter_context(tc.tile_pool(name="outT", bufs=1))
        fin_pool = ctx.enter_context(tc.tile_pool(name="fin", bufs=2))
        # PSUM budget (8 banks): ppsum 2x[64,512] = 2 banks (proj psum,
        # also vtrans/finals scratch), spsum 2x[128,1024] = 4 banks
        # (scores double-buffer), opsum 2 banks (warmup scratch, then the
        # per-pass [65,1024] out.T accumulator).
        ppsum = ctx.enter_context(tc.tile_pool(name="ppsum", bufs=2, space="PSUM"))
        spsum = ctx.enter_context(tc.tile_pool(name="spsum", bufs=2, space="PSUM"))
        opsum = ctx.enter_context(tc.tile_pool(name="opsum", bufs=1, space="PSUM"))

        # ---- constants (DMA'd first) ----
        wT_sb = const_pool.tile([128, MC * WB], BF16, tag="wt")
        nc.sync.dma_start(wT_sb[:], wT[:])
        bias_sb = const_pool.tile([128, 3], F32, tag="bias")
        nc.sync.dma_start(bias_sb[:], bias[:])
        # preload the ACT exp table while DMAs stream
        scratch = const_pool.tile([DF, 1], F32, tag="scratch")
        nc.scalar.activation(scratch[:], bias_sb[0:DF, 0:1], EXP)
        ident = const_pool.tile([128, 128], BF16, tag="ident")
        make_identity(nc, ident[:])
        identf = const_pool.tile([128, 128], F32, tag="identf")
        make_identity(nc, identf[:])

        # ---- PE warm-up: ~7us of dummy matmuls so the HAM clock gate
        # opens (1.2 -> 2.4 GHz) and stays open until the first real
        # matmul's input DMA lands ----
        warm = opsum.tile([DF, 128], F32, tag="po")
        for _ in range(96):
            nc.tensor.matmul(warm[:], ident[:, 0:DF], ident[:], start=True, stop=True)

        # ---- input DMAs, interleaved q first then k/v alternating ----
        def load_block(x_dram, i):
            t = xin_pool.tile([128, MC * NI], BF16, tag="xin")
            nc.sync.dma_start(t[:], x_dram[:, i * MC * NI:(i + 1) * MC * NI])
            return t

        q_tiles = [load_block(xq, i) for i in range(NBQ)]
        kv_tiles = {}
        for i in range(NBK):
            kv_tiles[("k", i)] = load_block(xk, i)
            kv_tiles[("v", i)] = load_block(xv, i)

        # ---- persistent activations ----
        # q/k projections land duplicated in both partition halves so the
        # score matmuls can run pair-wise on independent 64-row PE tiles
        qT_sb = act_pool.tile([128, SQ], BF16, tag="qT")
        kT_sb = act_pool.tile([128, S], BF16, tag="kT")
        vT_sb = act_pool.tile([DF, S], BF16, tag="vT")
        v_sb = act_pool.tile([128, JC * (DF + 1)], BF16, tag="v")  # [128, 32*65]
        nc.gpsimd.memset(v_sb[:], 1.0)  # col DF of every block stays 1.0

        def w_slice(mc_i, which):
            # which: 0 = [wq|wq], 1 = [wk|wk] (128-wide dup), 2 = wv (64)
            o = mc_i * WB + which * 2 * DF
            return wT_sb[:, o:o + (2 * DF if which < 2 else DF)]

        def project_block(x_tile, i, which, dest_sb, bias_col):
            """One 512-column projection block accumulated over 8 m-chunks."""
            rows = 2 * DF if which < 2 else DF
            ps = ppsum.tile([rows, NI], F32, tag="ps")
            for mc_i in range(MC):
                nc.tensor.matmul(
                    ps[:], w_slice(mc_i, which), x_tile[:, mc_i * NI:(mc_i + 1) * NI],
                    start=(mc_i == 0), stop=(mc_i == MC - 1),
                )
            nc.vector.tensor_scalar_add(
                dest_sb[:, i * NI:(i + 1) * NI], ps[:],
                bias_sb[0:rows, bias_col:bias_col + 1])

        # ---- q projection up front ----
        for i in range(NBQ):
            project_block(q_tiles[i], i, 0, qT_sb, 0)

        # pass-B exp results are computed during pass A and parked in SBUF
        ptb_sb = act_pool.tile([128, JC * IP], BF16, tag="ptb")  # 8 MB

        def attn_pair(jc0, poA):
            """Scores + exp for BOTH i-halves of TWO key chunks; the two
            chunks' score matmuls run on independent 64-row PE tiles
            (partitions 0-63 / 64-127 of the duplicated qT/kT), so they
            stream concurrently. attn@v for i-half A follows immediately;
            i-half B's exp output parks in ptb_sb."""
            for ipass in range(2):
                io = ipass * IP
                ss0 = spsum.tile([128, IP], F32, tag="ss", name="ss0")
                ss1 = spsum.tile([128, IP], F32, tag="ss", name="ss1")
                sss = [ss0, ss1]
                for ii in range(IP // NI):
                    for t in range(2):
                        jc = jc0 + t
                        p0 = t * DF
                        nc.tensor.matmul(
                            sss[t][:, ii * NI:(ii + 1) * NI],
                            kT_sb[p0:p0 + DF, jc * 128:(jc + 1) * 128],
                            qT_sb[p0:p0 + DF, io + ii * NI:io + (ii + 1) * NI],
                            start=True, stop=True,
                        )
                for t in range(2):
                    jc = jc0 + t
                    if ipass == 0:
                        pts = pt_pool.tile([128, IP], BF16, tag="pt")
                    else:
                        pts = ptb_sb[:, jc * IP:(jc + 1) * IP]
                    nc.scalar.activation(pts[:], sss[t][:], EXP, scale=0.125)
                    if ipass == 0:
                        for ii in range(IP // NI):
                            nc.tensor.matmul(
                                poA[:, ii * NI:(ii + 1) * NI],
                                v_sb[:, jc * (DF + 1):(jc + 1) * (DF + 1)],
                                pts[:, ii * NI:(ii + 1) * NI],
                                start=(jc == 0), stop=(jc == JC - 1),
                            )

        def finals_chunk(ipass, outT_sb, ob, c):
            pf = ppsum.tile([128, DF + 1], F32, tag="ps")
            nc.tensor.transpose(
                pf[:], outT_sb[:, c * 128:(c + 1) * 128],
                identf[0:DF + 1, 0:DF + 1])
            rcp = fin_pool.tile([128, 1], F32, tag="rcp")
            nc.vector.reciprocal(rcp[:], pf[:, DF:DF + 1])
            nc.vector.tensor_scalar_mul(ob[:, c, :], pf[:, 0:DF], rcp[:])

        def finals_store(ipass, ob):
            # one strided DMA for all 1024 rows of this i-half
            nc.sync.dma_start(
                out[ipass * IP:(ipass + 1) * IP, :].rearrange(
                    "(c p) f -> p c f", p=128),
                ob[:])

        # ---- pass A: k/v projection interleaved with scores/exp for both
        # i-halves + attn@v for i-half A ----
        poA = opsum.tile([DF + 1, IP], F32, tag="po")
        for kb in range(NBK):
            project_block(kv_tiles[("k", kb)], kb, 1, kT_sb, 1)
            project_block(kv_tiles[("v", kb)], kb, 2, vT_sb, 2)
            for jc in range(4 * kb, 4 * kb + 4):
                pv = ppsum.tile([128, DF], BF16, tag="ps")
                nc.tensor.transpose(
                    pv[:], vT_sb[:, jc * 128:(jc + 1) * 128], ident[0:DF, 0:DF])
                nc.vector.tensor_copy(
                    v_sb[:, jc * (DF + 1):jc * (DF + 1) + DF], pv[:])
            for jc0 in range(4 * kb, 4 * kb + 4, 2):
                attn_pair(jc0, poA)

        # ---- pass B: attn@v for i-half B from parked exp outputs; pass A
        # finals are interleaved to fill PE gaps. ----
        outT_A = outT_pool.tile([DF + 1, IP], F32, tag="outT")
        obA = fin_pool.tile([128, IP // 128, DF], F32, tag="ob")
        nc.vector.tensor_copy(outT_A[:], poA[:])
        poB = opsum.tile([DF + 1, IP], F32, tag="po")
        for jc in range(JC):
            for ii in range(IP // NI):
                nc.tensor.matmul(
                    poB[:, ii * NI:(ii + 1) * NI],
                    v_sb[:, jc * (DF + 1):(jc + 1) * (DF + 1)],
                    ptb_sb[:, jc * IP + ii * NI:jc * IP + (ii + 1) * NI],
                    start=(jc == 0), stop=(jc == JC - 1),
                )
            if jc % 4 == 3:
                finals_chunk(0, outT_A, obA, jc // 4)
        finals_store(0, obA)

        outT_B = outT_pool.tile([DF + 1, IP], F32, tag="outT")
        obB = fin_pool.tile([128, IP // 128, DF], F32, tag="ob")
        nc.vector.tensor_copy(outT_B[:], poB[:])
        for c in range(IP // 128):
            finals_chunk(1, outT_B, obB, c)
        finals_store(1, obB)


_COMPILED = None


def get_compiled():
    global _COMPILED
    if _COMPILED is None:
        nc = bacc.Bacc("TRN2", target_bir_lowering=False, debug=False,
                       enable_asserts=False, num_devices=NCORES)
        with tile.TileContext(nc) as tc:
            build_kernel(tc)
        nc.compile()
        _COMPILED = nc
    return _COMPILED


def _to_block_major(xT):
    """[DM, s_len] -> [128, nblk*MC*NI]: 512-col blocks, m-chunk-major inside."""
    s_len = xT.shape[1]
    nblk = s_len // NI
    # (mc, p, blk, s) -> (p, blk, mc, s)
    return np.ascontiguousarray(
        xT.reshape(MC, 128, nblk, NI).transpose(1, 2, 0, 3).reshape(128, nblk * MC * NI))


def make_in_maps(queries, keys, values, Wq, bq, Wk, bk, Wv, bv):
    queries = np.asarray(queries, dtype=np.float32)
    keys = np.asarray(keys, dtype=np.float32)
    values = np.asarray(values, dtype=np.float32)
    WqT, WkT, WvT = np.asarray(Wq).T, np.asarray(Wk).T, np.asarray(Wv).T
    wT_full = np.concatenate([WqT, WqT, WkT, WkT, WvT], axis=1)  # [DM, 320]
    wT_host = np.ascontiguousarray(
        wT_full.reshape(MC, 128, WB).transpose(1, 0, 2).reshape(128, MC * WB)
    ).astype(NP_BF16)
    bias64 = np.stack(
        [np.asarray(bq), np.asarray(bk), np.asarray(bv)], axis=1
    ).astype(np.float32)
    bias_host = np.concatenate([bias64, bias64], axis=0)  # [128, 3]

    in_maps = []
    for c in range(NCORES):
        b, h = c // 2, c % 2
        in_maps.append({
            "xq": _to_block_major(queries[b, h * SQ:(h + 1) * SQ, :].T).astype(NP_BF16),
            "xk": _to_block_major(keys[b].T).astype(NP_BF16),
            "xv": _to_block_major(values[b].T).astype(NP_BF16),
            "wT": wT_host, "bias": bias_host,
        })
    return in_maps


def assemble(results):
    out = np.zeros((B, S, DF), dtype=np.float32)
    for c in range(NCORES):
        b, h = c // 2, c % 2
        out[b, h * SQ:(h + 1) * SQ, :] = results[c]["out"]
    return out


def kernel(**inputs):
    nc = get_compiled()
    in_maps = make_in_maps(**inputs)
    res = run_bass_kernel_spmd(nc, in_maps, core_ids=list(range(NCORES)))
    return assemble(res.results)
